# revision 1
# baseline (speedup 1.0000x reference)
"""Trainium2 Bass kernel for nn_Attention_68298569941449.

Computes, for hidden_states [B=4, N=1024, D=1024], mask [B, N]:
    q = hs @ Wq.T ; k = hs @ Wk.T          (per-head split, H=16, hd=64)
    probs = softmax(q k^T / 64)            [B, H, N, N]
    nz = (mask == 0)
    out = g1*diag(nz) + g2*probs - g3*outer(nz,nz)/sum(nz)   [B, H, N, N]

Sharding: 64 (batch, head) pairs over 8 cores -> core c handles batch
c//2 and heads (c%2)*8 .. (c%2)*8+8.  No collectives.  Host marshals
per-core transposed bf16 operands; device does everything else.
"""

import numpy as np
import ml_dtypes
from contextlib import ExitStack

import concourse.bass as bass
import concourse.mybir as mybir
import concourse.tile as tile
from concourse import bacc
from concourse.bass_utils import run_bass_kernel_spmd
from concourse.masks import make_identity

B = 4
NT = 1024          # tokens / patches
DIM = 1024
NH = 16            # total heads
HD = 64            # head dim
NHL = 8            # heads per core
QD = NHL * HD      # local q/k output dims = 512
P = 128
KC = DIM // P      # 8 contraction chunks
RT = NT // P       # 8 row tiles
SCALE = 1.0 / 64.0

F32 = mybir.dt.float32
BF16 = mybir.dt.bfloat16
I32 = mybir.dt.int32
AX = mybir.AxisListType
ALU = mybir.AluOpType
ACTF = mybir.ActivationFunctionType

_CACHE = {}


def _build():
    nc = bacc.Bacc()
    hsT = nc.declare_dram_parameter("hsT", [DIM, NT], BF16, isOutput=False)
    wqT = nc.declare_dram_parameter("wqT", [DIM, QD], BF16, isOutput=False)
    wkT = nc.declare_dram_parameter("wkT", [DIM, QD], BF16, isOutput=False)
    mask = nc.declare_dram_parameter("mask", [NT], I32, isOutput=False)
    g = nc.declare_dram_parameter("g", [1, 3], F32, isOutput=False)
    out = nc.declare_dram_parameter("out", [NHL, NT, NT], F32, isOutput=True)

    with tile.TileContext(nc) as tc, ExitStack() as ctx:
        singles = ctx.enter_context(tc.tile_pool(name="singles", bufs=1))
        ppool = ctx.enter_context(tc.tile_pool(name="ps", bufs=3, space="PSUM"))
        epool = ctx.enter_context(tc.tile_pool(name="e", bufs=3))
        opool = ctx.enter_context(tc.tile_pool(name="o", bufs=4))
        small = ctx.enter_context(tc.tile_pool(name="small", bufs=4))

        # ---- resident loads -------------------------------------------------
        sb_hsT = singles.tile([P, KC, NT], BF16)
        for kc in range(KC):
            nc.sync.dma_start(out=sb_hsT[:, kc, :], in_=hsT[kc * P:(kc + 1) * P, :])
        sb_wqT = singles.tile([P, KC, QD], BF16)
        sb_wkT = singles.tile([P, KC, QD], BF16)
        for kc in range(KC):
            nc.sync.dma_start(out=sb_wqT[:, kc, :], in_=wqT[kc * P:(kc + 1) * P, :])
            nc.sync.dma_start(out=sb_wkT[:, kc, :], in_=wkT[kc * P:(kc + 1) * P, :])

        ident = singles.tile([P, P], F32)
        make_identity(nc, ident)

        # mask in two layouts: per-partition column [P, RT] and row [1, NT]
        m_pc = singles.tile([P, RT], I32)
        nc.sync.dma_start(out=m_pc, in_=mask[:].rearrange("(a p) -> p a", p=P))
        m_row = singles.tile([1, NT], I32)
        nc.sync.dma_start(out=m_row, in_=mask[:].rearrange("(a n) -> a n", a=1))

        # gammas: g1, g2 broadcast over partitions; g3 on partition 0
        g_row = singles.tile([1, 3], F32)
        nc.sync.dma_start(out=g_row, in_=g[:])
        gap = g[:]
        g1b = singles.tile([P, 1], F32)
        g2b = singles.tile([P, 1], F32)
        nc.gpsimd.dma_start(
            out=g1b, in_=bass.AP(tensor=gap.tensor, offset=0, ap=[[0, P], [1, 1]])
        )
        nc.gpsimd.dma_start(
            out=g2b, in_=bass.AP(tensor=gap.tensor, offset=1, ap=[[0, P], [1, 1]])
        )

        # nz in both layouts
        nz_col = singles.tile([P, RT], F32)
        nc.vector.tensor_scalar(nz_col, m_pc, 0, None, ALU.is_equal)
        nz_colg1 = singles.tile([P, RT], F32)
        nc.vector.tensor_scalar(nz_colg1, nz_col, g1b, None, ALU.mult)
        nz_row = singles.tile([1, NT], F32)
        nc.vector.tensor_scalar(nz_row, m_row, 0, None, ALU.is_equal)

        nnz = small.tile([1, 1], F32)
        nc.vector.tensor_reduce(nnz, nz_row, axis=AX.X, op=ALU.add)
        inv_nnz = small.tile([1, 1], F32)
        nc.vector.reciprocal(inv_nnz, nnz)
        # u_scale = -g3 * inv_nnz  (partition 0 only)
        u_scale = small.tile([1, 1], F32)
        nc.vector.tensor_scalar(
            u_scale, inv_nnz, g_row[0:1, 2:3], -1.0, ALU.mult, ALU.mult
        )
        u_row = singles.tile([1, NT], F32)
        nc.vector.tensor_scalar(u_row, nz_row, u_scale, None, ALU.mult)

        # ---- A = g1*diag(nz) - g3*outer(nz, nz)/nnz,  [P, RT, NT] ----------
        sb_A = singles.tile([P, RT, NT], F32)
        for rt in range(RT):
            psA = ppool.tile([P, NT], F32, tag="ps")
            for hf in range(2):
                nc.tensor.matmul(
                    psA[:, hf * 512:(hf + 1) * 512],
                    lhsT=u_row[0:1, rt * P:(rt + 1) * P],
                    rhs=nz_row[0:1, hf * 512:(hf + 1) * 512],
                    start=True,
                    stop=True,
                )
            nc.scalar.copy(out=sb_A[:, rt, :], in_=psA)
            # overwrite the 128-wide diagonal block: ident * (g1*nz[p]) + outer
            nc.vector.scalar_tensor_tensor(
                out=sb_A[:, rt, rt * P:(rt + 1) * P],
                in0=ident,
                scalar=nz_colg1[:, rt:rt + 1],
                in1=psA[:, rt * P:(rt + 1) * P],
                op0=ALU.mult,
                op1=ALU.add,
            )

        # ---- projections: qT/kT [P, 4, NT] bf16 (partition = qdim % 128) ---
        sb_qT = singles.tile([P, QD // P, NT], BF16)
        sb_kT = singles.tile([P, QD // P, NT], BF16)
        for w_sb, dst in ((sb_wqT, sb_qT), (sb_wkT, sb_kT)):
            for pt in range(QD // P):
                ps = ppool.tile([P, NT], F32, tag="ps")
                for hf in range(2):
                    for kc in range(KC):
                        nc.tensor.matmul(
                            ps[:, hf * 512:(hf + 1) * 512],
                            lhsT=w_sb[:, kc, pt * P:(pt + 1) * P],
                            rhs=sb_hsT[:, kc, hf * 512:(hf + 1) * 512],
                            start=(kc == 0),
                            stop=(kc == KC - 1),
                        )
                nc.scalar.copy(out=dst[:, pt, :], in_=ps)

        # ---- main loop: per (head, row-tile) -------------------------------
        for h in range(NHL):
            pt, po = h // 2, (h % 2) * HD
            for rt in range(RT):
                psS = ppool.tile([P, NT], F32, tag="ps")
                for hf in range(2):
                    nc.tensor.matmul(
                        psS[:, hf * 512:(hf + 1) * 512],
                        lhsT=sb_qT[po:po + HD, pt, rt * P:(rt + 1) * P],
                        rhs=sb_kT[po:po + HD, pt, hf * 512:(hf + 1) * 512],
                        start=True,
                        stop=True,
                    )
                e = epool.tile([P, NT], F32, tag="e")
                sums = small.tile([P, 1], F32, tag="sums")
                nc.scalar.activation(
                    out=e, in_=psS, func=ACTF.Exp, scale=SCALE, accum_out=sums
                )
                inv = small.tile([P, 1], F32, tag="inv")
                nc.vector.reciprocal(inv, sums)
                inv2 = small.tile([P, 1], F32, tag="inv2")
                nc.vector.tensor_scalar(inv2, inv, g2b, None, ALU.mult)
                o = opool.tile([P, NT], F32, tag="o")
                nc.vector.scalar_tensor_tensor(
                    out=o,
                    in0=e,
                    scalar=inv2,
                    in1=sb_A[:, rt, :],
                    op0=ALU.mult,
                    op1=ALU.add,
                )
                nc.sync.dma_start(out=out[h, rt * P:(rt + 1) * P, :], in_=o)

    nc.compile()
    return nc


def _get_nc():
    if "nc" not in _CACHE:
        _CACHE["nc"] = _build()
    return _CACHE["nc"]


def kernel(hidden_states, attention_mask, Wq, Wk, gamma_1, gamma_2, gamma_3,
           _trace=False):
    hs = np.asarray(hidden_states, dtype=np.float32)
    am = np.asarray(attention_mask, dtype=np.int32)
    Wq = np.asarray(Wq, dtype=np.float32)
    Wk = np.asarray(Wk, dtype=np.float32)
    g = np.array(
        [[float(gamma_1), float(gamma_2), float(gamma_3)]], dtype=np.float32
    )

    nc = _get_nc()
    bf16 = ml_dtypes.bfloat16
    in_maps = []
    for c in range(8):
        b, hg = c // 2, c % 2
        in_maps.append(
            {
                "hsT": np.ascontiguousarray(hs[b].T).astype(bf16),
                "wqT": np.ascontiguousarray(Wq[hg * QD:(hg + 1) * QD, :].T).astype(bf16),
                "wkT": np.ascontiguousarray(Wk[hg * QD:(hg + 1) * QD, :].T).astype(bf16),
                "mask": np.ascontiguousarray(am[b]),
                "g": g,
            }
        )
    res = run_bass_kernel_spmd(nc, in_maps, core_ids=list(range(8)), trace=_trace)
    out = np.empty((B, NH, NT, NT), np.float32)
    for c in range(8):
        b, hg = c // 2, c % 2
        out[b, hg * NHL:(hg + 1) * NHL] = res.results[c]["out"]
    if _trace:
        return out, res
    return out


# revision 3
# speedup vs baseline: 1.1364x; 1.1364x over previous
"""Trainium2 Bass kernel for nn_Attention_68298569941449.

Computes, for hidden_states [B=4, N=1024, D=1024], mask [B, N]:
    q = hs @ Wq.T ; k = hs @ Wk.T          (per-head split, H=16, hd=64)
    probs = softmax(q k^T / 64)            [B, H, N, N]
    nz = (mask == 0)
    out = g1*diag(nz) + g2*probs - g3*outer(nz,nz)/sum(nz)   [B, H, N, N]

Sharding: 64 (batch, head) pairs over 8 cores -> core c handles batch
c//2 and heads (c%2)*8 .. (c%2)*8+8.  No collectives.  Host marshals
per-core transposed bf16 operands; device does everything else.

Per-core schedule: mask/A setup first (warms PE while the 3 big input
DMAs stream), then a software pipeline over head-pair groups pt=0..3:
proj(pt) runs on PE while the epilogue stream (exp on ACT, normalize+add
on DVE/GpSimd, 512KB output DMAs) drains heads of pt-1.
"""

import numpy as np
import ml_dtypes
from contextlib import ExitStack

import concourse.bass as bass
import concourse.mybir as mybir
import concourse.tile as tile
from concourse import bacc
from concourse.bass_utils import run_bass_kernel_spmd
from concourse.masks import make_identity

B = 4
NT = 1024          # tokens / patches
DIM = 1024
NH = 16            # total heads
HD = 64            # head dim
NHL = 8            # heads per core
QD = NHL * HD      # local q/k output dims = 512
P = 128
KC = DIM // P      # 8 contraction chunks
RT = NT // P       # 8 row tiles
NPT = QD // P      # 4 projection partition-tiles (2 heads each)
SCALE = 1.0 / 64.0

F32 = mybir.dt.float32
BF16 = mybir.dt.bfloat16
I32 = mybir.dt.int32
AX = mybir.AxisListType
ALU = mybir.AluOpType
ACTF = mybir.ActivationFunctionType

# epilogue row-tiles handled by gpsimd instead of DVE (per head).
# NOTE: TRN2 Pool engine has no TensorScalarPtr opcode -> must stay empty.
GPS_RT = ()

_CACHE = {}


def _build():
    nc = bacc.Bacc()
    hsT = nc.declare_dram_parameter("hsT", [DIM, NT], BF16, isOutput=False)
    wqT = nc.declare_dram_parameter("wqT", [DIM, QD], BF16, isOutput=False)
    wkT = nc.declare_dram_parameter("wkT", [DIM, QD], BF16, isOutput=False)
    mask = nc.declare_dram_parameter("mask", [NT], I32, isOutput=False)
    g = nc.declare_dram_parameter("g", [1, 3], F32, isOutput=False)
    out = nc.declare_dram_parameter("out", [NHL, NT, NT], F32, isOutput=True)

    with tile.TileContext(nc) as tc, ExitStack() as ctx:
        singles = ctx.enter_context(tc.tile_pool(name="singles", bufs=1))
        ppool = ctx.enter_context(tc.tile_pool(name="ps", bufs=4, space="PSUM"))
        epool = ctx.enter_context(tc.tile_pool(name="e", bufs=5))
        opool = ctx.enter_context(tc.tile_pool(name="o", bufs=6))
        small = ctx.enter_context(tc.tile_pool(name="small", bufs=4))

        # ---- tiny inputs on the gpsimd (SWDGE) ring ------------------------
        m_pc = singles.tile([P, RT], I32)
        nc.gpsimd.dma_start(out=m_pc, in_=mask[:].rearrange("(a p) -> p a", p=P))
        m_row = singles.tile([1, NT], I32)
        nc.gpsimd.dma_start(out=m_row, in_=mask[:].rearrange("(a n) -> a n", a=1))
        g_row = singles.tile([1, 3], F32)
        nc.gpsimd.dma_start(out=g_row, in_=g[:])
        gap = g[:]
        g1b = singles.tile([P, 1], F32)
        g2b = singles.tile([P, 1], F32)
        nc.gpsimd.dma_start(
            out=g1b, in_=bass.AP(tensor=gap.tensor, offset=0, ap=[[0, P], [1, 1]])
        )
        nc.gpsimd.dma_start(
            out=g2b, in_=bass.AP(tensor=gap.tensor, offset=1, ap=[[0, P], [1, 1]])
        )

        # ---- big inputs: one DMA each on the sync (HWDGE) ring -------------
        sb_hsT = singles.tile([P, KC, NT], BF16)
        nc.sync.dma_start(
            out=sb_hsT, in_=hsT[:, :].rearrange("(kc p) t -> p kc t", p=P)
        )
        sb_wqT = singles.tile([P, KC, QD], BF16)
        nc.sync.dma_start(
            out=sb_wqT, in_=wqT[:, :].rearrange("(kc p) q -> p kc q", p=P)
        )
        sb_wkT = singles.tile([P, KC, QD], BF16)
        nc.sync.dma_start(
            out=sb_wkT, in_=wkT[:, :].rearrange("(kc p) q -> p kc q", p=P)
        )

        ident = singles.tile([P, P], F32)
        make_identity(nc, ident)

        # ---- nz vectors ----------------------------------------------------
        nz_col = singles.tile([P, RT], F32)
        nc.vector.tensor_scalar(nz_col, m_pc, 0, None, ALU.is_equal)
        nz_colg1 = singles.tile([P, RT], F32)
        nc.vector.tensor_scalar(nz_colg1, nz_col, g1b, None, ALU.mult)
        nz_row = singles.tile([1, NT], F32)
        nc.vector.tensor_scalar(nz_row, m_row, 0, None, ALU.is_equal)

        nnz = small.tile([1, 1], F32)
        nc.vector.tensor_reduce(nnz, nz_row, axis=AX.X, op=ALU.add)
        inv_nnz = small.tile([1, 1], F32)
        nc.vector.reciprocal(inv_nnz, nnz)
        u_scale = small.tile([1, 1], F32)  # -g3 / nnz   (partition 0)
        nc.vector.tensor_scalar(
            u_scale, inv_nnz, g_row[0:1, 2:3], -1.0, ALU.mult, ALU.mult
        )
        u_row = singles.tile([1, NT], F32)
        nc.vector.tensor_scalar(u_row, nz_row, u_scale, None, ALU.mult)

        # ---- A = g1*diag(nz) - g3*outer(nz, nz)/nnz,  [P, RT, NT] ----------
        # (rank-1 outer product on PE; also warms the PE clock gate while the
        #  big input DMAs are still in flight)
        sb_A = singles.tile([P, RT, NT], F32)
        for rt in range(RT):
            psA = ppool.tile([P, NT], F32, tag="ps")
            for hf in range(2):
                nc.tensor.matmul(
                    psA[:, hf * 512:(hf + 1) * 512],
                    lhsT=u_row[0:1, rt * P:(rt + 1) * P],
                    rhs=nz_row[0:1, hf * 512:(hf + 1) * 512],
                    start=True,
                    stop=True,
                )
            nc.scalar.copy(out=sb_A[:, rt, :], in_=psA)
            # overwrite the 128-wide diagonal block: ident * (g1*nz[p]) + outer
            nc.vector.scalar_tensor_tensor(
                out=sb_A[:, rt, rt * P:(rt + 1) * P],
                in0=ident,
                scalar=nz_colg1[:, rt:rt + 1],
                in1=psA[:, rt * P:(rt + 1) * P],
                op0=ALU.mult,
                op1=ALU.add,
            )

        sb_qT = singles.tile([P, NPT, NT], BF16)
        sb_kT = singles.tile([P, NPT, NT], BF16)

        def proj(pt):
            """q,k projections for partition-tile pt (heads 2pt, 2pt+1)."""
            for w_sb, dst, ceng in (
                (sb_wqT, sb_qT, nc.scalar),
                (sb_wkT, sb_kT, nc.vector),
            ):
                ps = ppool.tile([P, NT], F32, tag="ps")
                for hf in range(2):
                    for kc in range(KC):
                        nc.tensor.matmul(
                            ps[:, hf * 512:(hf + 1) * 512],
                            lhsT=w_sb[:, kc, pt * P:(pt + 1) * P],
                            rhs=sb_hsT[:, kc, hf * 512:(hf + 1) * 512],
                            start=(kc == 0),
                            stop=(kc == KC - 1),
                        )
                if ceng is nc.scalar:
                    ceng.copy(out=dst[:, pt, :], in_=ps)
                else:
                    ceng.tensor_copy(out=dst[:, pt, :], in_=ps)

        def head_stream(h):
            """scores + softmax + epilogue + output DMA for local head h."""
            pt, po = h // 2, (h % 2) * HD
            for half in range(2):
                sums = small.tile([P, 4], F32, tag="sums")
                es = []
                for rtl in range(4):
                    rt = half * 4 + rtl
                    psS = ppool.tile([P, NT], F32, tag="ps")
                    for hf in range(2):
                        nc.tensor.matmul(
                            psS[:, hf * 512:(hf + 1) * 512],
                            lhsT=sb_qT[po:po + HD, pt, rt * P:(rt + 1) * P],
                            rhs=sb_kT[po:po + HD, pt, hf * 512:(hf + 1) * 512],
                            start=True,
                            stop=True,
                        )
                    e = epool.tile([P, NT], F32, tag="e")
                    nc.scalar.activation(
                        out=e,
                        in_=psS,
                        func=ACTF.Exp,
                        scale=SCALE,
                        accum_out=sums[:, rtl:rtl + 1],
                    )
                    es.append(e)
                inv = small.tile([P, 4], F32, tag="inv")
                nc.vector.reciprocal(inv, sums)
                inv2 = small.tile([P, 4], F32, tag="inv2")
                nc.vector.tensor_scalar(inv2, inv, g2b, None, ALU.mult)
                for rtl in range(4):
                    rt = half * 4 + rtl
                    eng = nc.gpsimd if rt in GPS_RT else nc.vector
                    o = opool.tile([P, NT], F32, tag="o")
                    eng.scalar_tensor_tensor(
                        out=o,
                        in0=es[rtl],
                        scalar=inv2[:, rtl:rtl + 1],
                        in1=sb_A[:, rt, :],
                        op0=ALU.mult,
                        op1=ALU.add,
                    )
                    nc.sync.dma_start(out=out[h, rt * P:(rt + 1) * P, :], in_=o)

        # software pipeline: proj(pt) overlaps the stream of heads 2pt-2..2pt-1
        proj(0)
        for pt in range(1, NPT):
            proj(pt)
            head_stream(2 * (pt - 1))
            head_stream(2 * (pt - 1) + 1)
        head_stream(2 * (NPT - 1))
        head_stream(2 * (NPT - 1) + 1)

    nc.compile()
    return nc


def _get_nc():
    if "nc" not in _CACHE:
        _CACHE["nc"] = _build()
    return _CACHE["nc"]


def kernel(hidden_states, attention_mask, Wq, Wk, gamma_1, gamma_2, gamma_3,
           _trace=False):
    hs = np.asarray(hidden_states, dtype=np.float32)
    am = np.asarray(attention_mask, dtype=np.int32)
    Wq = np.asarray(Wq, dtype=np.float32)
    Wk = np.asarray(Wk, dtype=np.float32)
    g = np.array(
        [[float(gamma_1), float(gamma_2), float(gamma_3)]], dtype=np.float32
    )

    nc = _get_nc()
    bf16 = ml_dtypes.bfloat16
    in_maps = []
    for c in range(8):
        b, hg = c // 2, c % 2
        in_maps.append(
            {
                "hsT": np.ascontiguousarray(hs[b].T).astype(bf16),
                "wqT": np.ascontiguousarray(Wq[hg * QD:(hg + 1) * QD, :].T).astype(bf16),
                "wkT": np.ascontiguousarray(Wk[hg * QD:(hg + 1) * QD, :].T).astype(bf16),
                "mask": np.ascontiguousarray(am[b]),
                "g": g,
            }
        )
    res = run_bass_kernel_spmd(nc, in_maps, core_ids=list(range(8)), trace=_trace)
    out = np.empty((B, NH, NT, NT), np.float32)
    for c in range(8):
        b, hg = c // 2, c % 2
        out[b, hg * NHL:(hg + 1) * NHL] = res.results[c]["out"]
    if _trace:
        return out, res
    return out


# revision 4
# speedup vs baseline: 1.4679x; 1.2917x over previous
"""Trainium2 Bass kernel for nn_Attention_68298569941449.

Computes, for hidden_states [B=4, N=1024, D=1024], mask [B, N]:
    q = hs @ Wq.T ; k = hs @ Wk.T          (per-head split, H=16, hd=64)
    probs = softmax(q k^T / 64)            [B, H, N, N]
    nz = (mask == 0)
    out = g1*diag(nz) + g2*probs - g3*outer(nz,nz)/sum(nz)   [B, H, N, N]

Sharding: 64 (batch, head) pairs over 8 cores -> core c handles batch
c//2 and heads (c%2)*8 .. (c%2)*8+8.  No collectives.

Precision scheme: projections run in fp8e4m3 DoubleRow (host passes hs.T
and 16*W.T as fp8; the 16*16 product scale folds into the exp scale
2^-14).  Scores run in fp8 DoubleRow with the second contraction slot
carrying the q-side quantization residual (q16 - fp8(q16)) while k's
slot is a stride-0 broadcast — so q enters the scores matmul at ~fp16
precision for free.  A-matrix outer product in bf16.  Everything
accumulates in fp32 PSUM; softmax + epilogue in fp32.

Per-core schedule: mask/A setup first (warms the PE while inputs
stream), then a software pipeline: proj(pt) on PE overlaps the epilogue
stream (exp on ACT, normalize+add on DVE, 512KB output DMAs) of the
previous head pair.
"""

import numpy as np
from contextlib import ExitStack

import concourse.bass as bass
import concourse.mybir as mybir
import concourse.tile as tile
from concourse import bacc
from concourse.bass_utils import run_bass_kernel_spmd
from concourse.masks import make_identity

B = 4
NT = 1024          # tokens / patches
DIM = 1024
NH = 16            # total heads
HD = 64            # head dim
NHL = 8            # heads per core
QD = NHL * HD      # local q/k output dims = 512
P = 128
KC = DIM // P      # 8 contraction chunks of 128
RT = NT // P       # 8 row tiles
NPT = QD // P      # 4 projection partition-tiles (2 heads each)
W_PRESCALE = 16.0
SCALE = 1.0 / (64.0 * W_PRESCALE * W_PRESCALE)   # 2^-14, exact

F32 = mybir.dt.float32
BF16 = mybir.dt.bfloat16
FP8 = mybir.dt.float8e4
I32 = mybir.dt.int32
AX = mybir.AxisListType
ALU = mybir.AluOpType
ACTF = mybir.ActivationFunctionType
DR = mybir.MatmulPerfMode.DoubleRow

_CACHE = {}


def _slot_broadcast(ap2d):
    """[P, N] AP -> [P, 2, N] AP with a stride-0 middle (k-slot) axis."""
    return bass.AP(
        tensor=ap2d.tensor,
        offset=ap2d.offset,
        ap=[ap2d.ap[0], [0, 2], ap2d.ap[1]],
    )


def _build():
    nc = bacc.Bacc()
    hsT = nc.declare_dram_parameter("hsT", [DIM, NT], FP8, isOutput=False)
    wqT = nc.declare_dram_parameter("wqT", [DIM, QD], FP8, isOutput=False)
    wkT = nc.declare_dram_parameter("wkT", [DIM, QD], FP8, isOutput=False)
    mask = nc.declare_dram_parameter("mask", [NT], I32, isOutput=False)
    g = nc.declare_dram_parameter("g", [1, 3], F32, isOutput=False)
    out = nc.declare_dram_parameter("out", [NHL, NT, NT], F32, isOutput=True)

    with tile.TileContext(nc) as tc, ExitStack() as ctx:
        singles = ctx.enter_context(tc.tile_pool(name="singles", bufs=1))
        ppool = ctx.enter_context(tc.tile_pool(name="ps", bufs=4, space="PSUM"))
        epool = ctx.enter_context(tc.tile_pool(name="e", bufs=5))
        opool = ctx.enter_context(tc.tile_pool(name="o", bufs=6))
        small = ctx.enter_context(tc.tile_pool(name="small", bufs=4))

        # ---- tiny inputs first on the sync ring, gammas on gpsimd ----------
        m_row = singles.tile([1, NT], I32)
        nc.sync.dma_start(out=m_row, in_=mask[:].rearrange("(a n) -> a n", a=1))
        m_pc = singles.tile([P, RT], I32)
        nc.sync.dma_start(out=m_pc, in_=mask[:].rearrange("(a p) -> p a", p=P))
        g_row = singles.tile([1, 3], F32)
        nc.gpsimd.dma_start(out=g_row, in_=g[:])
        gap = g[:]
        g1b = singles.tile([P, 1], F32)
        g2b = singles.tile([P, 1], F32)
        nc.gpsimd.dma_start(
            out=g1b, in_=bass.AP(tensor=gap.tensor, offset=0, ap=[[0, P], [1, 1]])
        )
        nc.gpsimd.dma_start(
            out=g2b, in_=bass.AP(tensor=gap.tensor, offset=1, ap=[[0, P], [1, 1]])
        )

        # ---- big inputs: one DMA each --------------------------------------
        sb_hsT = singles.tile([P, KC, NT], FP8)
        nc.sync.dma_start(
            out=sb_hsT, in_=hsT[:, :].rearrange("(kc p) t -> p kc t", p=P)
        )
        sb_wqT = singles.tile([P, KC, QD], FP8)
        nc.sync.dma_start(
            out=sb_wqT, in_=wqT[:, :].rearrange("(kc p) q -> p kc q", p=P)
        )
        sb_wkT = singles.tile([P, KC, QD], FP8)
        nc.sync.dma_start(
            out=sb_wkT, in_=wkT[:, :].rearrange("(kc p) q -> p kc q", p=P)
        )

        ident = singles.tile([P, P], F32)
        make_identity(nc, ident)

        # ---- nz vectors ----------------------------------------------------
        nz_col = singles.tile([P, RT], F32)
        nc.vector.tensor_scalar(nz_col, m_pc, 0, None, ALU.is_equal)
        nz_colg1 = singles.tile([P, RT], F32)
        nc.vector.tensor_scalar(nz_colg1, nz_col, g1b, None, ALU.mult)
        nz_row = singles.tile([1, NT], BF16)   # exact 0/1 values
        nc.vector.tensor_scalar(nz_row, m_row, 0, None, ALU.is_equal)

        nnz = small.tile([1, 1], F32)
        nc.vector.tensor_reduce(nnz, nz_row, axis=AX.X, op=ALU.add)
        inv_nnz = small.tile([1, 1], F32)
        nc.vector.reciprocal(inv_nnz, nnz)
        u_scale = small.tile([1, 1], F32)  # -g3 / nnz   (partition 0)
        nc.vector.tensor_scalar(
            u_scale, inv_nnz, g_row[0:1, 2:3], -1.0, ALU.mult, ALU.mult
        )
        u_row = singles.tile([1, NT], BF16)
        nc.vector.tensor_scalar(u_row, nz_row, u_scale, None, ALU.mult)

        # ---- A = g1*diag(nz) - g3*outer(nz, nz)/nnz,  [P, RT, NT] ----------
        # (rank-1 outer product on PE; also warms the PE clock gate while the
        #  big input DMAs are still in flight)
        sb_A = singles.tile([P, RT, NT], F32)
        for rt in range(RT):
            psA = ppool.tile([P, NT], F32, tag="ps")
            for hf in range(2):
                nc.tensor.matmul(
                    psA[:, hf * 512:(hf + 1) * 512],
                    lhsT=u_row[0:1, rt * P:(rt + 1) * P],
                    rhs=nz_row[0:1, hf * 512:(hf + 1) * 512],
                    start=True,
                    stop=True,
                )
            nc.scalar.copy(out=sb_A[:, rt, :], in_=psA)
            # overwrite the 128-wide diagonal block: ident * (g1*nz[p]) + outer
            nc.vector.scalar_tensor_tensor(
                out=sb_A[:, rt, rt * P:(rt + 1) * P],
                in0=ident,
                scalar=nz_colg1[:, rt:rt + 1],
                in1=psA[:, rt * P:(rt + 1) * P],
                op0=ALU.mult,
                op1=ALU.add,
            )

        # qT8: [P, NPT, 2, NT] fp8 — slot 0 = fp8(q16), slot 1 = residual
        # kT8: [P, NPT, NT] fp8 — k-slot axis broadcast at matmul time
        sb_qT = singles.tile([P, NPT, 2, NT], FP8)
        sb_kT = singles.tile([P, NPT, NT], FP8)

        def proj(pt):
            """q,k projections for partition-tile pt (heads 2pt, 2pt+1)."""
            for w_sb, is_q in ((sb_wqT, True), (sb_wkT, False)):
                ps = ppool.tile([P, NT], F32, tag="ps")
                for hf in range(2):
                    for j in range(KC // 2):
                        nc.tensor.matmul(
                            ps[:, hf * 512:(hf + 1) * 512],
                            lhsT=w_sb[:, 2 * j:2 * j + 2, pt * P:(pt + 1) * P],
                            rhs=sb_hsT[:, 2 * j:2 * j + 2,
                                       hf * 512:(hf + 1) * 512],
                            start=(j == 0),
                            stop=(j == KC // 2 - 1),
                            perf_mode=DR,
                        )
                if is_q:
                    nc.scalar.copy(out=sb_qT[:, pt, 0, :], in_=ps)
                    # q-side fp8 residual into the second contraction slot
                    nc.vector.tensor_sub(
                        sb_qT[:, pt, 1, :], ps, sb_qT[:, pt, 0, :]
                    )
                else:
                    nc.scalar.copy(out=sb_kT[:, pt, :], in_=ps)

        def head_stream(h):
            """scores + softmax + epilogue + output DMA for local head h."""
            pt, po = h // 2, (h % 2) * HD
            for half in range(2):
                sums = small.tile([P, 4], F32, tag="sums")
                es = []
                for rtl in range(4):
                    rt = half * 4 + rtl
                    psS = ppool.tile([P, NT], F32, tag="ps")
                    for hf in range(2):
                        nc.tensor.matmul(
                            psS[:, hf * 512:(hf + 1) * 512],
                            lhsT=sb_qT[po:po + HD, pt, :, rt * P:(rt + 1) * P],
                            rhs=_slot_broadcast(
                                sb_kT[po:po + HD, pt,
                                      hf * 512:(hf + 1) * 512]
                            ),
                            start=True,
                            stop=True,
                            perf_mode=DR,
                        )
                    e = epool.tile([P, NT], F32, tag="e")
                    nc.scalar.activation(
                        out=e,
                        in_=psS,
                        func=ACTF.Exp,
                        scale=SCALE,
                        accum_out=sums[:, rtl:rtl + 1],
                    )
                    es.append(e)
                inv = small.tile([P, 4], F32, tag="inv")
                nc.vector.reciprocal(inv, sums)
                inv2 = small.tile([P, 4], F32, tag="inv2")
                nc.vector.tensor_scalar(inv2, inv, g2b, None, ALU.mult)
                for rtl in range(4):
                    rt = half * 4 + rtl
                    o = opool.tile([P, NT], F32, tag="o")
                    nc.vector.scalar_tensor_tensor(
                        out=o,
                        in0=es[rtl],
                        scalar=inv2[:, rtl:rtl + 1],
                        in1=sb_A[:, rt, :],
                        op0=ALU.mult,
                        op1=ALU.add,
                    )
                    nc.sync.dma_start(out=out[h, rt * P:(rt + 1) * P, :], in_=o)

        # software pipeline: proj(pt) overlaps the stream of heads 2pt-2..2pt-1
        proj(0)
        for pt in range(1, NPT):
            proj(pt)
            head_stream(2 * (pt - 1))
            head_stream(2 * (pt - 1) + 1)
        head_stream(2 * (NPT - 1))
        head_stream(2 * (NPT - 1) + 1)

    nc.compile()
    return nc


def _get_nc():
    if "nc" not in _CACHE:
        _CACHE["nc"] = _build()
    return _CACHE["nc"]


def kernel(hidden_states, attention_mask, Wq, Wk, gamma_1, gamma_2, gamma_3,
           _trace=False):
    hs = np.asarray(hidden_states, dtype=np.float32)
    am = np.asarray(attention_mask, dtype=np.int32)
    Wq = np.asarray(Wq, dtype=np.float32)
    Wk = np.asarray(Wk, dtype=np.float32)
    g = np.array(
        [[float(gamma_1), float(gamma_2), float(gamma_3)]], dtype=np.float32
    )

    nc = _get_nc()
    fp8 = mybir.dt.np(FP8)
    in_maps = []
    for c in range(8):
        b, hg = c // 2, c % 2
        wq = (W_PRESCALE * Wq[hg * QD:(hg + 1) * QD, :]).T
        wk = (W_PRESCALE * Wk[hg * QD:(hg + 1) * QD, :]).T
        in_maps.append(
            {
                "hsT": np.ascontiguousarray(hs[b].T).astype(fp8),
                "wqT": np.ascontiguousarray(wq).astype(fp8),
                "wkT": np.ascontiguousarray(wk).astype(fp8),
                "mask": np.ascontiguousarray(am[b]),
                "g": g,
            }
        )
    res = run_bass_kernel_spmd(nc, in_maps, core_ids=list(range(8)), trace=_trace)
    out = np.empty((B, NH, NT, NT), np.float32)
    for c in range(8):
        b, hg = c // 2, c % 2
        out[b, hg * NHL:(hg + 1) * NHL] = res.results[c]["out"]
    if _trace:
        return out, res
    return out
